# revision 19
# baseline (speedup 1.0000x reference)
"""Trainium2 Bass kernel for BasicCNN+LSTM (conv3x3+ReLU+GAP -> custom LSTM scan).

Self-contained: hardcodes shapes/sharding. Data-parallel over batch B=8 across
8 NeuronCores; each core processes one batch element end-to-end, the host
gathers the 8 [1,32] results.

Per-core device pipeline (per frame t of 24):
  - The frame's host-prepacked stack [36, 6272] bf16 is split by column into
    two SBUF bands (cols 0-3135 at partitions 0-35, rest at 64-99), so both
    PE row groups (h0/h64) work on ONE frame; every LDWEIGHTS is pulled ahead
    during the other row group's stream (ldw-opt dedupe is broken in this
    walrus, so the reload must be hidden, not removed).
  - Conv as K=36, M=128 matmuls, N=448, into pool-rotated 2-bank PSUM tiles
    [128, 1024] (bufs=3). The stationary's 96 real columns sit at cols
    32:128 (zero-pad at 0:32) so conv outputs live at PARTITIONS 32:128 -
    that makes every downstream AP 32-aligned (walrus requires 32-aligned
    partition bases and identical partition ranges for all engine operands).
  - One fused ReLU(+bias)+GAP drain per tile over the [96(base32), 2, 448]
    view, alternating ScalarE activation(Relu, accum_out) and VectorE
    tensor_scalar((x max 0) add 0, accum_out). Both run at their documented
    hardware rate ((172+FD)/1.2 resp (120+FD)/0.96); keeping each engine's
    instruction mix uniform avoids the inter-instruction read-write bubble.
  - GAP finalize: pairwise add tree on the otherwise idle GpSimd; the final
    add writes bf16 straight into scan-input rows z[32:128] (fsum form; the
    pixel-pair fold + 1/(H*W) are absorbed into the scan stationary).
  - Scan step: 4 matmuls pg[0:32, g] = sfull[:, 32g:32g+32].T @ z[128,1]
    at tile_position (0,0) put gate1/gate2/gate3/cell pre-acts in 4 COLUMNS
    of one [32,4] PSUM tile, so ONE ScalarE Tanh covers all gates AND
    tanh(cell_{t-1}) and every recurrence operand sits at partitions 0:32.
    z = [celld(0:32); fsum(32:128)]; sfull carries all gate scaling
    (tanh-form sigmoid: W1/W2 slots halved => sigmoid via (tanh(x/2)+1)/2),
    the celld/2 cell-read scaling, and 0.5*I cell-passthrough columns.
    The recurrence runs on GpSimd as pure tensor_tensor ops against const
    ones/halfs tiles (walrus rejects tensor_scalar variants on Pool):
    h2=celld*tc; a4=(ga1+1)*h2; b2=(ga2+1)*t3; celld'=a4/2+b2; stage celld'
    bf16 into z_{t+1}[0:32]. celld is tracked at 2x scale. The reference's
    state-order swap bug (z concatenates prev new_cell; x1 multiplies prev
    new_hidden) is reproduced faithfully.

Known dead ends (measured): GpSimd has no PSUM port; tile_position col 96
(quadrant 3) crashes the exec unit and bass only allows col 0/64; walrus
enforces samePartitionsAll + 32-aligned partition bases on engine ops;
folding the GAP tree into per-column K=96 N=1 PE matmuls costs ~70us of PE
(N=1 matmul floor ~120ns + non-deduped LDWEIGHTS) and perturbs the drain
engines' instruction mix (+20% per drain).
"""
import sys
if '/opt/trn_rl_repo' not in sys.path:
    sys.path.insert(0, '/opt/trn_rl_repo')

import numpy as np
import ml_dtypes

import concourse.bass as bass
import concourse.mybir as mybir
import concourse.tile as tile
from concourse.vector_clock import ScopedClock
from concourse.bass_utils import run_bass_kernel_spmd

# ---------------------------------------------------------------- constants
B, T, H, W, C, F, U = 8, 24, 112, 112, 3, 48, 32
JA = 56            # vertical pixel-pair blocks (112 rows / 2)
KP = 36            # stack partitions: 3 c x 3 dx x 4 window rows
M = 96             # 2 pixels x 48 filters (real rows; stationary padded to 128)
NQ = 448
FREE = JA * W      # stack free size per partition (elements)

FP32 = mybir.dt.float32
BF16 = mybir.dt.bfloat16

LAST_RESULTS = None  # BassKernelResults of the most recent run (for test.py)

# ------------------------------------------------- TileContext drain patch
# The container's walrus rejects >1 semaphore wait per instruction; Tile's
# kernel-tail drain aggregates all end-of-kernel waits onto one Drain.
# Spread them across single-wait NOPs on the sync engine instead.
def _patched_drain_and_barrier(self, tick_clock, wait_clock):
    nc = self.nc
    probe = nc.sync.nop(nofuse=True, hint="tail_waits")
    wait_clock.add_sem_waits(probe.ins, ScopedClock({None: tick_clock.global_clock}))
    waits = list(probe.ins.sync_info.on_wait or [])
    if len(waits) > 1:
        probe.ins.sync_info.on_wait = waits[:1]
        for i in range(1, len(waits)):
            extra = nc.sync.nop(nofuse=True, hint=f"tail_waits_{i}")
            si = extra.ins.sync_info
            if si is None:
                extra.ins.sync_info = mybir.SyncInfo(on_wait=[waits[i]], on_update=[])
            else:
                si.on_wait = [waits[i]]
    nc.sync.drain()
    nc.all_engine_barrier()
    popped = nc._tile_sem_poison_stack.pop()
    assert popped is self._sem_poison
    nc.clear_and_free_semaphores(list(self.sems.allocated().values()))
    nc.all_engine_barrier()


tile.TileContext._drain_and_barrier = _patched_drain_and_barrier

# Same walrus restriction for regular instructions: spill extra sem waits
# onto preceding same-engine NOPs at commit time.
_orig_commit = tile.TileContext._commit_instruction


def _patched_commit(self, inst, *args, **kwargs):
    si = getattr(inst, 'sync_info', None)
    if si is not None and si.on_wait and len(si.on_wait) > 1 \
            and inst.engine != mybir.EngineType.Unassigned:
        waits = list(si.on_wait)
        si.on_wait = waits[-1:]
        for w in waits[:-1]:
            nop = mybir.InstNoOp(
                name=self.nc.get_next_instruction_name(),
                ins=[], outs=[], bass_is_fusable=False)
            nop.engine = inst.engine
            nop.sync_info = mybir.SyncInfo(on_wait=[w], on_update=[])
            _orig_commit(self, nop, *args, **kwargs)
    return _orig_commit(self, inst, *args, **kwargs)


tile.TileContext._commit_instruction = _patched_commit


# ------------------------------------------------------------- device code
def _build_bass(use_cbias=True, use_gbias=False):
    _build_bass.use_cbias = use_cbias
    _build_bass.use_gbias = use_gbias
    nc = bass.Bass('TRN2', target_bir_lowering=False, debug=False)

    xin = nc.dram_tensor('xin', [T, KP, FREE], BF16, kind='ExternalInput')
    smat_d = nc.dram_tensor('smat', [KP, 128], BF16, kind='ExternalInput')
    cbias_d = nc.dram_tensor('cbias', [M, 1], FP32, kind='ExternalInput')
    sfull_d = nc.dram_tensor('sfull', [128, 128], BF16, kind='ExternalInput')
    gbias4_d = nc.dram_tensor('gbias4', [U, 4], FP32, kind='ExternalInput')
    outh_d = nc.dram_tensor('outh', [U, 1], FP32, kind='ExternalOutput')

    Relu = mybir.ActivationFunctionType.Relu
    Tanh = mybir.ActivationFunctionType.Tanh
    Amax = mybir.AluOpType.max
    Aadd = mybir.AluOpType.add
    Amul = mybir.AluOpType.mult

    with tile.TileContext(nc) as tc:
        const = tc.alloc_tile_pool(name='const', bufs=1)
        state = tc.alloc_tile_pool(name='state', bufs=1)
        stackp = tc.alloc_tile_pool(name='stack', bufs=6)
        psum = tc.alloc_tile_pool(name='psum', bufs=4, space='PSUM')
        gs = tc.alloc_tile_pool(name='gs', bufs=4)
        zp = tc.alloc_tile_pool(name='zp', bufs=6)
        ga_pool = tc.alloc_tile_pool(name='ga', bufs=4)
        tmp = tc.alloc_tile_pool(name='tmp', bufs=10)

        # conv stationary first - the very first matmul needs only this
        sc_all = const.tile([128, 128], BF16, tag='sc')
        for s in range(2):
            nc.sync.dma_start(sc_all[64 * s:64 * s + KP, :], smat_d[:])
        cbias = const.tile([M, 1], FP32, tag='cb')
        sfullF = const.tile([M, 128], BF16, tag='sfF')
        sfullC = const.tile([U, 128], BF16, tag='sfC')
        gbias4 = const.tile([U, 4], FP32, tag='gb4') if use_gbias else None
        ztv = None
        if use_cbias:
            zt = const.tile([M, 3 * NQ], BF16, tag='zt')
            nc.vector.memset(zt[:], 0.0)
            ztv = zt.rearrange("p (r n) -> p r n", r=3)[:, 0:2, :]
        ones = const.tile([U, 1], FP32, tag='ones')
        nc.vector.memset(ones[:], 1.0)
        halfs = const.tile([U, 1], FP32, tag='halfs')
        nc.vector.memset(halfs[:], 0.5)

        # persistent scan state: celld = 2*new_cell, fp32, partitions 0:32
        celld = state.tile([U, 1], FP32, tag='celld')
        nc.vector.memset(celld[:], 0.0)

        frames = [None] * T
        fsums = [None] * T
        cbs = [None] * T

        def load_late_consts():
            # issued AFTER frame 0/1 stack DMAs: nothing here is needed
            # before the first drain/scan, so don't delay the pipeline head
            nc.sync.dma_start(cbias[:], cbias_d[:])
            nc.sync.dma_start(sfullF[:], sfull_d[0:M])
            nc.sync.dma_start(sfullC[:], sfull_d[M:128])
            if use_gbias:
                nc.sync.dma_start(gbias4[:], gbias4_d[:])
            # pre-warm the ACT spline table during the DMA wait (the first
            # Relu otherwise pays the ~1.3us ACT_TABLE_LOAD mid-pipeline)
            nc.scalar.activation(ones[:], ones[:], Relu)

        def get_frame(t):
            # frame t split by column: cols 0-3135 -> band h0, rest -> h64
            if frames[t] is None:
                rt = stackp.tile([128, FREE // 2], BF16, tag='stk')
                nc.sync.dma_start(rt[0:KP, :], xin[t][:, 0:FREE // 2])
                nc.sync.dma_start(rt[64:64 + KP, :], xin[t][:, FREE // 2:])
                frames[t] = rt
            return frames[t]

        def get_cb(t):
            if cbs[t] is None:
                cbs[t] = zp.tile([U, 1], BF16, tag='cb', name='cb')
            return cbs[t]

        def emit_tile(rt, k7, eng, gsum, col):
            # one pool tile per 448-col chunk pair: band h0 chunk at
            # bank-aligned offset 0, band h64 chunk at 512; one drain
            ps = psum.tile([128, 1024], FP32, tag='ps', name='ps')
            for b in range(2):
                band = rt[64 * b:64 * b + KP, :]
                lhsT = sc_all[64 * b:64 * b + KP, :]
                nc.tensor.matmul(ps[:, b * 512:b * 512 + NQ], lhsT,
                                 band[:, k7 * NQ:(k7 + 1) * NQ],
                                 start=True, stop=True,
                                 tile_position=(64 * b, 0))
            psv = ps[0:M, :].rearrange("p (b n) -> p b n", b=2)[:, :, 0:NQ]
            if eng == 'A':
                nc.scalar.activation(psv, psv, Relu, bias=cbias[:],
                                     accum_out=gsum[:, col:col + 1])
            elif _build_bass.use_cbias:
                # (x + bias) max 0 with summing accum (two tensor sources)
                nc.vector.scalar_tensor_tensor(
                    out=psv, in0=psv, scalar=cbias[:], in1=ztv,
                    op0=Aadd, op1=Amax, accum_out=gsum[:, col:col + 1])
            else:
                # zero bias: (x max 0) add 0; accum reduces with op1 (add),
                # single tensor source -> full DVE rate
                nc.vector.tensor_scalar(
                    out=psv, in0=psv, scalar1=0.0, scalar2=0.0,
                    op0=Amax, op1=Aadd, accum_out=gsum[:, col:col + 1])

        def emit_fold(t, gsumA, gsumB, cols):
            # GAP finalize: pairwise add tree on the idle GpSimd -> bf16 fsum
            cs = [gsumA[:, c:c + 1] for c in range(cols[0])] + \
                 [gsumB[:, c:c + 1] for c in range(cols[1])]
            fsum = zp.tile([M, 1], BF16, tag='fsum', name='fsum')
            while len(cs) > 1:
                nxt = []
                for i in range(0, len(cs) - 1, 2):
                    o = fsum if len(cs) == 2 else \
                        tmp.tile([M, 1], FP32, tag='fst', name='fst')
                    nc.gpsimd.tensor_add(o[:], cs[i], cs[i + 1])
                    nxt.append(o)
                if len(cs) % 2:
                    nxt.append(cs[-1])
                cs = nxt
            fsums[t] = fsum

        pgs = [None] * T

        def emit_scan_mm(t, g):
            # gate preacts via pairs of accumulating matmuls (fsum part K=96
            # + celld part K=32) into 4 COLUMNS of one [32,4] PSUM tile; one
            # pair per k7 slot so the PE never stalls on a serialized wedge.
            # g==3 is the cell passthrough (celld part only).  z-hidden part
            # = prev new_cell (reference's state-order swap bug); x1
            # multiplier = prev new_hidden = 0.5*celld*tanh(cell).
            if g == 0:
                # pg borrows a rotation slot of the conv PSUM pool (frees 2
                # banks -> conv pool deepens to bufs=4, loosening the
                # mm->drain->mm ring)
                pgs[t] = psum.tile([128, 1024], FP32, tag='ps', name='ps')
            pg = pgs[t][0:U, 0:4]
            cb = get_cb(t)
            if g < 3:
                nc.tensor.matmul(pg[:, g:g + 1], sfullF[:, g * U:(g + 1) * U],
                                 fsums[t][:], start=True, stop=False,
                                 tile_position=(0, 0))
                nc.tensor.matmul(pg[:, g:g + 1], sfullC[:, g * U:(g + 1) * U],
                                 cb[:], start=False, stop=True,
                                 tile_position=(0, 0))
            else:
                nc.tensor.matmul(pg[:, 3:4], sfullC[:, 96:128], cb[:],
                                 start=True, stop=True, tile_position=(0, 0))

        def emit_scan_post(t):
            pg = pgs[t][0:U, 0:4]
            if use_gbias:
                nc.vector.tensor_add(pg[:], pg[:], gbias4[:])
            ga = ga_pool.tile([U, 4], FP32, tag='ga')
            nc.scalar.activation(ga[:], pg[:], Tanh)
            # recurrence on GpSimd (tensor_tensor/tcopy only work there)
            h2 = tmp.tile([U, 1], FP32, tag='h2')
            nc.gpsimd.tensor_mul(h2[:], celld[:], ga[:, 3:4])   # 2*h_prev
            g1p = tmp.tile([U, 1], FP32, tag='g1p')
            nc.gpsimd.tensor_add(g1p[:], ga[:, 0:1], ones[:])
            a4 = tmp.tile([U, 1], FP32, tag='a4')
            nc.gpsimd.tensor_mul(a4[:], g1p[:], h2[:])          # 4*x1
            a2h = tmp.tile([U, 1], FP32, tag='a2h')
            nc.gpsimd.tensor_mul(a2h[:], a4[:], halfs[:])       # 2*x1
            g2p = tmp.tile([U, 1], FP32, tag='g2p')
            nc.gpsimd.tensor_add(g2p[:], ga[:, 1:2], ones[:])
            b2 = tmp.tile([U, 1], FP32, tag='b2')
            nc.gpsimd.tensor_mul(b2[:], g2p[:], ga[:, 2:3])     # 2*x2*x3
            nc.gpsimd.tensor_add(celld[:], a2h[:], b2[:])       # 2*new_cell
            if t < T - 1:
                nc.gpsimd.tensor_copy(get_cb(t + 1)[:], celld[:])
            else:
                # output: new_hidden = cell*tanh(cell) = 0.5*celld*tanh(celld/2)
                tcl = tmp.tile([U, 1], FP32, tag='tcl')
                nc.scalar.activation(tcl[:], celld[:], Tanh, scale=0.5)
                outv = tmp.tile([U, 1], FP32, tag='outv')
                nc.vector.scalar_tensor_tensor(
                    out=outv[:], in0=celld[:], scalar=0.5, in1=tcl[:],
                    op0=Amul, op1=Amul)
                nc.sync.dma_start(outh_d[:], outv[:])

        LAG = 2  # frames of lag between a frame's conv and its scan step
        nc.vector.memset(get_cb(0)[:], 0.0)  # celld_{-1} = 0

        gsums = [None] * T
        for t in range(T):
            rt = get_frame(t)
            if t == 0:
                get_frame(1)
                load_late_consts()
            for tp in range(t + 2, min(t + 6, T)):
                get_frame(tp)    # prefetch up to 5 frames ahead
            # alternate drain engines; flip per frame to balance 4/3 -> 3.5
            pat = ('ADADADA', 'DADADAD')[t % 2]
            gsumA = gs.tile([M, 4], FP32, tag='gsumA', name='gsumA')
            gsumB = gs.tile([M, 4], FP32, tag='gsumB', name='gsumB')
            gsums[t] = (gsumA, gsumB)
            cols = [0, 0]
            for k7 in range(7):
                eng = pat[k7]
                ei = 0 if eng == 'A' else 1
                emit_tile(rt, k7, eng, gsumA if eng == 'A' else gsumB,
                          cols[ei])
                cols[ei] += 1
                # cols per frame: 'ADADADA' -> A=4,B=3; 'DADADAD' -> A=3,B=4
                # scan early in the frame so the serial chain (mms ->
                # tanh -> GpSimd recurrence) finishes mid-frame with slack
                # instead of pacing the frame; fold late (GpSimd order:
                # recurrence first, then fold for the next frame's scan)
                if t >= LAG and k7 < 2:
                    emit_scan_mm(t - LAG, 2 * k7)
                    emit_scan_mm(t - LAG, 2 * k7 + 1)
                if t >= LAG and k7 == 2:
                    emit_scan_post(t - LAG)
                if k7 == 5 and t >= 1:
                    ga_, gb_ = gsums[t - 1]
                    emit_fold(t - 1, ga_, gb_,
                              [4, 3] if (t - 1) % 2 == 0 else [3, 4])
        emit_fold(T - 1, *gsums[T - 1],
                  [4, 3] if (T - 1) % 2 == 0 else [3, 4])
        for t in range(T - LAG, T):
            for g in range(4):
                emit_scan_mm(t, g)
            emit_scan_post(t)

        for p in (tmp, ga_pool, zp, gs, psum, stackp, state, const):
            p.release()

    return nc


# -------------------------------------------------------------- host prep
def _prep_inputs(x, conv_w, conv_b, W1, b1, W2, b2, W3, b3):
    x = np.asarray(x, np.float32)
    conv_w = np.asarray(conv_w, np.float32)
    conv_b = np.asarray(conv_b, np.float32)

    xp = np.zeros((B, T, H + 2, W + 2, C), np.float32)
    xp[:, :, 1:H + 1, 1:W + 1, :] = x
    xin2 = np.empty((B, T, KP, JA, W), np.float32)
    rows = 2 * np.arange(JA)
    for c in range(3):
        for dx in range(3):
            for r in range(4):
                p = c * 12 + dx * 4 + r
                xin2[:, :, p] = np.moveaxis(
                    xp[:, :, rows + r, dx:dx + W, c], 0, 2)
    xin2 = xin2.reshape(B, T, KP, FREE).astype(ml_dtypes.bfloat16)

    smat = np.zeros((KP, 128), np.float32)
    for c in range(3):
        for dx in range(3):
            for r in range(4):
                p = c * 12 + dx * 4 + r
                for i in range(2):
                    dy = r - i
                    if 0 <= dy <= 2:
                        smat[p, i * F:(i + 1) * F] = conv_w[dy, dx, c, :]
    smat = smat.astype(ml_dtypes.bfloat16)
    cbias = np.concatenate([conv_b, conv_b]).reshape(M, 1).astype(np.float32)

    # scan stationaries: rows 0:96 = fsum part (pixel-pair dup + 1/(H*W)),
    # rows 96:128 = celld part.  Col blocks g*32+u: g in {0,1} sigmoid slots
    # (tanh-form => halved): 0.5*Wg[f,u]/(H*W) resp 0.25*Wg[48+j,u]; g=2
    # tanh slot: W3[f,u]/(H*W) resp 0.5*W3[48+j,u].  Cols 96+j of the celld
    # part: 0.5 identity (pg col 3 = cell => tanh(cell)).
    sfull = np.zeros((128, 128), np.float32)
    for g, Wg in enumerate([W1, W2, W3]):
        Wg = np.asarray(Wg, np.float32)
        half = 0.5 if g < 2 else 1.0
        for i in range(2):
            sfull[i * F:(i + 1) * F, g * U:(g + 1) * U] = \
                Wg[0:F, :] * (half / float(H * W))
        sfull[M + np.arange(U)[:, None], g * U + np.arange(U)[None, :]] = \
            Wg[F:F + U, :] * (half * 0.5)
    sfull[M:128, 96:128] = 0.5 * np.eye(U, dtype=np.float32)
    sfull = sfull.astype(ml_dtypes.bfloat16)

    gbias4 = np.zeros((U, 4), np.float32)
    gbias4[:, 0] = np.asarray(b1, np.float32) * 0.5
    gbias4[:, 1] = np.asarray(b2, np.float32) * 0.5
    gbias4[:, 2] = np.asarray(b3, np.float32)

    return xin2, smat, cbias, sfull, gbias4


# ------------------------------------------------------------------ kernel
def kernel(x, conv_w, conv_b, W1, b1, W2, b2, W3, b3, W4, b4):
    global LAST_RESULTS
    xin2, smat, cbias, sfull, gbias4 = _prep_inputs(
        x, conv_w, conv_b, W1, b1, W2, b2, W3, b3)

    nc = _build_bass(use_cbias=bool(np.any(cbias)),
                     use_gbias=bool(np.any(gbias4)))
    in_maps = [{
        'xin': np.ascontiguousarray(xin2[b]),
        'smat': smat,
        'cbias': cbias,
        'sfull': sfull,
        'gbias4': gbias4,
    } for b in range(B)]

    res = run_bass_kernel_spmd(nc, in_maps, core_ids=list(range(B)))
    LAST_RESULTS = res
    out = np.stack([res.results[b]['outh'][:, 0] for b in range(B)], axis=0)
    return out.astype(np.float32)
